# revision 40
# baseline (speedup 1.0000x reference)
"""Multi-head causal attention (B=2, S=2048, H=16, DH=64, D=1024) on 8 TRN2 cores.

Sharding: Megatron tensor-parallel over heads - core c owns heads {2c, 2c+1}:
  * column-slices of Wq/Wk/Wv (128 cols each) + bias slices,
  * row-slice of Wo (128 rows),
  * full hidden_states (pre-transposed on host to [D, B*S]).
Each core computes a partial output (its 2 heads through Wo rows) in fp16;
host sums the 8 partials and adds bo.

Single fused pipeline, engine-specialized:
  * Tensor: QKV matmuls, V transposes, score matmuls (two heads run
    concurrently on row-split PE tiles), causal masking (a -160 constant
    strict-lower-triangular matrix accumulated into the diagonal-chunk
    scores through the PE, so exp() kills masked positions - no elementwise
    mask ops), AV matmuls (ones-augmented V gives softmax denominators),
    output projection.  Attention blocks are emitted as soon as their
    K/V chunks exist, so QKV groups and outproj blocks fill the PE while
    the scalar engine's exp streams run, and the PE never idles long
    enough for the HAM clock gate to re-throttle.
  * Scalar (ACT): exp, plus half the outproj PSUM evacuations.
  * Vector (DVE): QKV bias evacuations, softmax normalize
    (copy den / reciprocal / multiplies), other half of outproj evacs.
  * GpSimd (Pool): denominator partition-broadcasts, vaug copies.
Per attention block, AV matmuls are issued one wave behind scores so the
in-order tensor queue never head-of-line blocks on an exp semaphore.
"""
import os
import sys

sys.path.insert(0, "/opt/trn_rl_repo")

DEBUG_TAPS = os.environ.get("KERNEL_DEBUG_TAPS") == "1"

from contextlib import ExitStack

import numpy as np

import concourse.bass as bass
import concourse.mybir as mybir
import concourse.tile as tile
from concourse import bacc
from concourse.bass_utils import run_bass_kernel_spmd

F32 = mybir.dt.float32
F16 = mybir.dt.float16
MM_DT = F16
MM_NP = np.float16

B, S, H, DH = 2, 2048, 16, 64
D = H * DH            # 1024
T = B * S             # 4096 tokens
NCORES = 8
HPC = H // NCORES     # 2 heads per core
KC = D // 128         # 8 contraction chunks
NG = T // 512         # 8 token groups for QKV (groups 0-3 = batch 0)
NQB = S // 256        # 8 query blocks per batch
NKV = T // 128        # 32 kv chunks of 128 tokens
EXPFN = mybir.ActivationFunctionType.Exp


def _body(nc, tc, ctx, t_in, t_dbg=None):
    xt, wq, wk, wv, wo, bq, bk, bv, maskw, ident, vones, po = t_in

    const = ctx.enter_context(tc.tile_pool(name="const", bufs=1))
    big = ctx.enter_context(tc.tile_pool(name="big", bufs=1))
    xtp = ctx.enter_context(tc.tile_pool(name="xtp", bufs=6))
    ep = ctx.enter_context(tc.tile_pool(name="ep", bufs=6))
    dnp = ctx.enter_context(tc.tile_pool(name="dnp", bufs=3))
    osp = ctx.enter_context(tc.tile_pool(name="osp", bufs=4))
    tpp = ctx.enter_context(tc.tile_pool(name="tpp", bufs=8))

    # PSUM: psS 2x[128,1024]f32 (4 banks) + psC 2x[65,2,256]f32 (2 banks)
    #       + psO 2x[128,512]f32 (2 banks) = 8 banks
    psS = ctx.enter_context(tc.tile_pool(name="psS", bufs=2, space="PSUM"))
    psC = ctx.enter_context(tc.tile_pool(name="psC", bufs=2, space="PSUM"))
    psO = ctx.enter_context(tc.tile_pool(name="psO", bufs=2, space="PSUM"))

    # ---- constants / weights in SBUF
    wq_s = const.tile([128, KC, 128], MM_DT, tag="wq")
    wk_s = const.tile([128, KC, 128], MM_DT, tag="wk")
    wv_s = const.tile([128, KC, 128], MM_DT, tag="wv")
    wo_s = const.tile([128, D], MM_DT, tag="wo")
    bq_s = const.tile([128, 1], F32, tag="bq")
    bk_s = const.tile([128, 1], F32, tag="bk")
    bv_s = const.tile([128, 1], F32, tag="bv")
    mk_s = const.tile([128, 384], MM_DT, tag="maskw")
    id_s = const.tile([128, 128], MM_DT, tag="ident")
    nc.scalar.dma_start(wv_s[:], wv[:])
    nc.scalar.dma_start(bv_s[:], bv[:])
    nc.scalar.dma_start(wq_s[:], wq[:])
    nc.scalar.dma_start(bq_s[:], bq[:])
    nc.scalar.dma_start(wk_s[:], wk[:])
    nc.scalar.dma_start(bk_s[:], bk[:])
    nc.gpsimd.dma_start(wo_s[:], wo[:])
    nc.gpsimd.dma_start(mk_s[:], maskw[:])
    nc.gpsimd.dma_start(id_s[:], ident[:])

    qT = big.tile([128, T], MM_DT, tag="qT")
    kT = big.tile([128, T], MM_DT, tag="kT")
    vT = big.tile([128, T], MM_DT, tag="vT")
    ctxT = big.tile([128, T], MM_DT, tag="ctxT")
    vaug = big.tile([128, NKV, HPC, 65], MM_DT, tag="vaug")
    # ones column of V_aug (softmax denominator trick)
    nc.gpsimd.dma_start(vaug[:, :, :, 64].rearrange("p c h -> p (c h)"), vones[:])

    woc = wo_s[:].rearrange("p (c n) -> p c n", c=KC)

    xgs = {}
    fetched = set()

    def fetch_x(g):
        fetched.add(g)
        # two half-tiles so the first matmul starts after half the fetch
        xga = xtp.tile([128, KC // 2, 512], MM_DT, tag="xt")
        nc.sync.dma_start(xga[:], xt[g, :, 0:KC // 2, :])
        xgb = xtp.tile([128, KC // 2, 512], MM_DT, tag="xt")
        nc.sync.dma_start(xgb[:], xt[g, :, KC // 2:KC, :])
        xgs[g] = (xga, xgb)

    pending_copies = []

    def flush_copies():
        while pending_copies:
            chunk, tpt = pending_copies.pop(0)
            nc.vector.tensor_copy(
                vaug[:, chunk, :, 0:64],
                tpt[:].rearrange("p (h d) -> p h d", h=HPC))

    def emit_qkv_group(g):
        xga, xgb = xgs.pop(g)
        cols = slice(g * 512, (g + 1) * 512)
        # V first so its evacuation (which feeds the XBAR transposes) is
        # issued as early as possible on the vector queue.
        for (w_s, b_s, dst) in ((wv_s, bv_s, vT), (wq_s, bq_s, qT),
                                (wk_s, bk_s, kT)):
            acc = psO.tile([128, 512], F32, tag="po")
            for k in range(KC):
                xk = xga[:, k, :] if k < KC // 2 else xgb[:, k - KC // 2, :]
                nc.tensor.matmul(acc[:], w_s[:, k, :], xk,
                                 start=(k == 0), stop=(k == KC - 1))
            nc.vector.tensor_scalar_add(dst[:, cols], acc[:], b_s[:])
        for gg in range(g + 1, min(g + 4, NG)):
            if gg not in xgs and gg not in fetched:
                fetch_x(gg)
        # V transpose into vaug for this group's 4 kv chunks (XBAR DMA
        # transpose into contiguous staging; the strided copy into vaug is
        # deferred so it never head-of-line blocks QKV evacuations).
        for chunk in range(g * 4, g * 4 + 4):
            tpt = tpp.tile([128, 128], MM_DT, tag="tp")
            nc.sync.dma_start_transpose(
                tpt[:], vT[:, chunk * 128:(chunk + 1) * 128])
            pending_copies.append((chunk, tpt))

    pending_norms = []

    def flush_norms():
        # normalize: ctxT[:, q] = ct[0:64] * (1 / ct[64]) broadcast.
        # Deferred so the next QKV group's PSUM evacuations (which gate the
        # tensor engine via psO pool rotation) enter the vector queue first.
        while pending_norms:
            qi, b, ct, qcols = pending_norms.pop(0)
            den = dnp.tile([1, 2, 256], F32, tag="den")
            nc.vector.tensor_copy(den[:], ct[64:65, :, :])
            denb = dnp.tile([64, 2, 256], F32, tag="denb")
            nc.gpsimd.partition_broadcast(denb[:], den[:])
            rden = dnp.tile([64, 2, 256], F32, tag="rden")
            nc.vector.reciprocal_approx_fast(rden[:], denb[:])
            for h in range(HPC):
                nc.vector.tensor_mul(ctxT[h * 64:(h + 1) * 64, qcols],
                                     ct[0:64, h, :], rden[:, h, :])
            if t_dbg is not None:
                blk = b * NQB + qi
                nc.sync.dma_start(t_dbg[0][blk], den[:])
                nc.sync.dma_start(t_dbg[1][blk], rden[0:1, :, :])

    def emit_attn_block(qi, b):
        flush_copies()
        flush_norms()
        qcols = slice(b * S + qi * 256, b * S + qi * 256 + 256)
        nch = 2 * (qi + 1)          # kv chunks of 128 for this q block
        nwaves = nch // 2
        ct = psC.tile([65, 2, 256], F32, tag="ct")
        pend = []                   # deferred AV work, one wave behind

        def emit_av(js, e):
            for h in range(HPC):
                for i, j in enumerate(js):
                    lhsT = vaug[:, b * (S // 128) + j, h, :]
                    # start=True clears has_written for the WHOLE psum bank;
                    # ct packs both heads in one bank, so only the very first
                    # AV matmul of the block may carry it (everything later
                    # accumulates, reading not-yet-written entries as zero).
                    first = (j == 0 and h == 0)
                    last = (j == nch - 1)
                    if j == nch - 1:    # odd diag: q first half all masked
                        nc.tensor.matmul(ct[:, h, 128:256], lhsT,
                                         e[:, 2 * h + i, 128:256],
                                         start=first, stop=last)
                    else:
                        nc.tensor.matmul(ct[:, h, :], lhsT,
                                         e[:, 2 * h + i, :],
                                         start=first, stop=last)

        for w in range(nwaves):
            js = (2 * w, 2 * w + 1)
            sp = psS.tile([128, 2 * HPC, 256], F32, tag="sp")
            for h in range(HPC):
                for i, j in enumerate(js):
                    kcols = slice(b * S + j * 128, b * S + j * 128 + 128)
                    diag = j >= nch - 2
                    nc.tensor.matmul(
                        sp[:, 2 * h + i, :],
                        kT[h * 64:(h + 1) * 64, kcols],
                        qT[h * 64:(h + 1) * 64, qcols],
                        start=True, stop=not diag)
                    if diag:
                        # accumulate -160 * strict-lower-tri: exp -> ~0
                        mof = 128 if j == nch - 1 else 0
                        nc.tensor.matmul(
                            sp[:, 2 * h + i, :], id_s[:],
                            mk_s[:, mof:mof + 256],
                            start=False, stop=True)
            e = ep.tile([128, 2 * HPC, 256], MM_DT, tag="e")
            nc.scalar.activation(e[:], sp[:], EXPFN, scale=0.125)
            if pend:
                emit_av(*pend.pop())
            pend.append((js, e))
        emit_av(*pend.pop())
        pending_norms.append((qi, b, ct, qcols))

    def emit_outproj(b, t):
        flush_norms()
        # output projection for one completed 512-token block, 2 chunks/step
        tg = b * (S // 512) + t
        tcols = slice(tg * 512, (tg + 1) * 512)
        for c in range(0, KC, 2):
            op = psS.tile([128, 2, 512], F32, tag="sp")
            for ci in range(2):
                nc.tensor.matmul(op[:, ci, :], woc[:, c + ci, :],
                                 ctxT[:, tcols], start=True, stop=True)
            ost = osp.tile([128, 2, 512], MM_DT, tag="ost")
            if c in (0, 4):
                nc.vector.tensor_copy(ost[:], op[:])
            else:
                nc.scalar.copy(ost[:], op[:])
            for ci in range(2):
                nc.gpsimd.dma_start(po[c + ci, :, tcols], ost[:, ci, :])

    # ---- fused schedule -------------------------------------------------
    # each attention block is emitted as soon as its K/V chunks and query
    # rows exist; outproj blocks trail their two attention blocks.
    fetch_x(0)

    emit_qkv_group(0)
    emit_qkv_group(1)
    emit_attn_block(0, 0)
    emit_qkv_group(2)
    emit_attn_block(1, 0)
    emit_attn_block(2, 0)
    emit_qkv_group(3)
    emit_attn_block(3, 0)
    emit_outproj(0, 0)
    emit_qkv_group(4)
    emit_attn_block(4, 0)
    emit_attn_block(0, 1)
    emit_qkv_group(5)
    emit_attn_block(5, 0)
    emit_attn_block(1, 1)
    emit_outproj(0, 1)
    emit_qkv_group(6)
    emit_attn_block(6, 0)
    emit_attn_block(2, 1)
    emit_outproj(0, 2)
    emit_qkv_group(7)
    emit_attn_block(7, 0)
    emit_attn_block(3, 1)
    emit_outproj(0, 3)
    emit_attn_block(4, 1)
    emit_outproj(1, 0)
    emit_attn_block(5, 1)
    emit_attn_block(6, 1)
    emit_outproj(1, 1)
    emit_attn_block(7, 1)
    emit_outproj(1, 2)
    emit_outproj(1, 3)

    if t_dbg is not None:
        dbp = ctx.enter_context(tc.tile_pool(name="dbp", bufs=1))
        for src, dst in ((qT, t_dbg[2]), (kT, t_dbg[3]), (ctxT, t_dbg[4])):
            tmp = dbp.tile([128, T], F32, tag="dbgt")
            nc.vector.tensor_copy(tmp[:], src[:])
            nc.sync.dma_start(dst[:], tmp[:])
        tmpv = dbp.tile([128, NKV * HPC * 65], F32, tag="dbgt")
        nc.vector.tensor_copy(
            tmpv[:], vaug[:].rearrange("p c h x -> p (c h x)"))
        nc.sync.dma_start(t_dbg[5][:], tmpv[:])


_NC = None


def _build():
    global _NC
    if _NC is not None:
        return _NC
    nc = bacc.Bacc("TRN2", target_bir_lowering=False, debug=False,
                   num_devices=NCORES)
    t_in = [
        nc.dram_tensor("xt", [NG, 128, KC, 512], MM_DT, kind="ExternalInput").ap(),
        nc.dram_tensor("wq", [128, KC, 128], MM_DT, kind="ExternalInput").ap(),
        nc.dram_tensor("wk", [128, KC, 128], MM_DT, kind="ExternalInput").ap(),
        nc.dram_tensor("wv", [128, KC, 128], MM_DT, kind="ExternalInput").ap(),
        nc.dram_tensor("wo", [128, D], MM_DT, kind="ExternalInput").ap(),
        nc.dram_tensor("bq", [128, 1], F32, kind="ExternalInput").ap(),
        nc.dram_tensor("bk", [128, 1], F32, kind="ExternalInput").ap(),
        nc.dram_tensor("bv", [128, 1], F32, kind="ExternalInput").ap(),
        nc.dram_tensor("maskw", [128, 384], MM_DT, kind="ExternalInput").ap(),
        nc.dram_tensor("ident", [128, 128], MM_DT, kind="ExternalInput").ap(),
        nc.dram_tensor("vones", [128, NKV * HPC], MM_DT,
                       kind="ExternalInput").ap(),
        nc.dram_tensor("po", [KC, 128, T], MM_DT, kind="ExternalOutput").ap(),
    ]
    t_dbg = None
    if DEBUG_TAPS:
        t_dbg = [
            nc.dram_tensor("dbg_den", [16, 1, 2, 256], F32, kind="ExternalOutput").ap(),
            nc.dram_tensor("dbg_rden", [16, 1, 2, 256], F32, kind="ExternalOutput").ap(),
            nc.dram_tensor("dbg_qT", [128, T], F32, kind="ExternalOutput").ap(),
            nc.dram_tensor("dbg_kT", [128, T], F32, kind="ExternalOutput").ap(),
            nc.dram_tensor("dbg_ctxT", [128, T], F32, kind="ExternalOutput").ap(),
            nc.dram_tensor("dbg_vaug", [128, NKV * HPC * 65], F32,
                           kind="ExternalOutput").ap(),
        ]
    with tile.TileContext(nc) as tc, ExitStack() as ctx:
        _body(nc, tc, ctx, t_in, t_dbg)
    nc.compile()
    _NC = nc
    return nc


def _in_maps(hidden_states, Wq, bq, Wk, bk, Wv, bv, Wo, bo):
    hid = np.asarray(hidden_states, dtype=np.float32).reshape(T, D)
    hidT = hid.T.astype(MM_NP)                       # [D, T]
    xt = np.ascontiguousarray(
        hidT.reshape(KC, 128, NG, 512).transpose(2, 1, 0, 3))
    # maskw[:, 0:256] masks cols 0:128 (even diag chunk); maskw[:, 128:384]
    # masks cols 128:256 (odd diag chunk). [kv, q]: masked when kv > q.
    maskw = np.zeros((128, 384), np.float32)
    maskw[:, 0:128] = np.tril(np.full((128, 128), -160.0), -1)
    maskw[:, 256:384] = maskw[:, 0:128]
    common = {
        "xt": xt,
        "maskw": maskw.astype(MM_NP),
        "ident": np.eye(128, dtype=MM_NP),
        "vones": np.ones((128, NKV * HPC), MM_NP),
    }
    maps = []
    for c in range(NCORES):
        cs = slice(c * 128, (c + 1) * 128)
        maps.append(dict(
            common,
            wq=np.ascontiguousarray(np.asarray(Wq)[:, cs].astype(MM_NP).reshape(KC, 128, 128).transpose(1, 0, 2)),
            wk=np.ascontiguousarray(np.asarray(Wk)[:, cs].astype(MM_NP).reshape(KC, 128, 128).transpose(1, 0, 2)),
            wv=np.ascontiguousarray(np.asarray(Wv)[:, cs].astype(MM_NP).reshape(KC, 128, 128).transpose(1, 0, 2)),
            wo=np.ascontiguousarray(np.asarray(Wo)[cs, :].astype(MM_NP)),
            bq=np.asarray(bq)[cs].reshape(128, 1).astype(np.float32),
            bk=np.asarray(bk)[cs].reshape(128, 1).astype(np.float32),
            bv=np.asarray(bv)[cs].reshape(128, 1).astype(np.float32),
        ))
    return maps


def kernel(hidden_states, Wq, bq, Wk, bk, Wv, bv, Wo, bo):
    nc = _build()
    maps = _in_maps(hidden_states, Wq, bq, Wk, bk, Wv, bv, Wo, bo)
    res = run_bass_kernel_spmd(nc, maps, list(range(NCORES))).results
    acc = np.zeros((KC, 128, T), np.float32)
    for r in res:
        acc += r["po"].astype(np.float32)
    outT = acc.reshape(D, T) + np.asarray(bo, np.float32).reshape(D, 1)
    return outT.T.reshape(B, S, D).astype(np.float32)
